# revision 21
# baseline (speedup 1.0000x reference)
"""Trainium2 Bass kernel for nn_GCNN_87668872446200 (aggregate-first design).

GCN identity used: with norm = dinv[src]*dinv[dst],
    h = lrelu(segsum(norm * (xW)[src] -> dst) + b)
      = lrelu((dinv[dst] * segsum((x*dinv)[src] -> dst)) @ W + b)
so the scatter runs on raw features (S matrices are exactly 0/1) and the
dense transform happens once per destination node (no replication).

Sharding: 8 cores x 1280 destination nodes (10 blocks of 128 dests,
degree-balanced node->block assignment), full F=1024 on every core.
Per core and branch:
  - dma_gather pulls xs = x*dinv rows (host-precast) for this core's edges
  - scatter-add realized as PE matmuls: S[128 edges, 128 dests] (0/1,
    host-built) x gathered[128 edges, 1024] accumulated in PSUM per block
  - agg -> (x dinv[dst] scale, cast) -> PE transpose -> aggT -> dense @ W_g
    (+bias via DVE, LeakyReLU via ACT) -> per-block pooling matmul
  - per-branch AllReduce of pooledT partials (branch 1's overlaps branch 2)
  - masif branch: 4 graphs/core; replicated dense head -> sigmoid -> [1, 32]

All 8 cores run ONE identical program; per-core variation is in input data.
"""
import numpy as np

# ---------------------------------------------------------------- constants
N_CORES = 8
P = 128
GRP = 6            # chunks per gather call (<=8 so num_idxs <= 1024)

N_NODES, N_EDGES, F_DIM, B_GRAPHS, L_MAS, C_MAS = 10000, 80000, 1024, 32, 800, 16

# dtype / perf config (module defaults = shipped config)
CFG_GDT = 'float8_e4m3'  # gather payload + S matrix dtype
CFG_WDT = 'float8_e4m3'  # dense (aggT, W_g) dtype
CFG_DR = True            # DoubleRow fp8 matmuls for scatter+dense
CFG_NQ = 2               # number of SWDGE queues for gathers
W_SCALE = 4.0            # W_g x4, dinv x1/4: keeps fp8 operands normal-range


class _Cfg:
    def __init__(self, n=N_NODES, e=N_EDGES, f=F_DIM, b=B_GRAPHS,
                 l=L_MAS, c=C_MAS):
        assert f % P == 0 and b == 32 and l % 80 == 0 and c % 2 == 0
        self.N, self.E, self.F, self.B, self.L, self.C = n, e, f, b, l, c
        self.NBLK = 10                          # dest blocks per core
        self.NBT = self.NBLK * N_CORES          # total dest blocks
        self.KC = f // P                        # contraction chunks (8)
        self.GPB = b // N_CORES                 # graphs per core for masif
        self.LW = l // 80                       # avg-pool window (10)
        self.LB = 8                             # l-blocks for masif layout
        self.LBS = l // self.LB                 # l-block size (100)
        assert self.LBS % self.LW == 0
        self.WPB = self.LBS // self.LW          # windows per l-block (10)


# ---------------------------------------------------------------- host prep
def _balance_nodes(deg, nbt):
    """Snake-assign nodes (sorted by degree desc) to nbt blocks of <=128,
    balancing per-block edge sums. Returns (blk[node], slot[node])."""
    n = deg.shape[0]
    order = np.argsort(-deg, kind='stable')
    blk = np.empty(n, np.int64)
    pos = np.arange(n)
    rnd = pos // nbt
    off = pos % nbt
    fwd = (rnd % 2) == 0
    blk[order] = np.where(fwd, off, nbt - 1 - off)
    slot = np.empty(n, np.int64)
    for b in range(nbt):
        sel = np.flatnonzero(blk == b)
        assert len(sel) <= P, f"block {b} has {len(sel)} nodes"
        slot[sel] = np.arange(len(sel))
    return blk, slot


def _edge_plan(cfg, edge_index, use_dr):
    """Build the shared chunk schedule + per-core gather idx / S streams."""
    row = np.asarray(edge_index[0]).astype(np.int64)
    col = np.asarray(edge_index[1]).astype(np.int64)
    loops = np.arange(cfg.N, dtype=np.int64)
    rows = np.concatenate([row, loops])
    cols = np.concatenate([col, loops])
    deg = np.bincount(cols, minlength=cfg.N).astype(np.float64)
    dinv = (1.0 / np.sqrt(deg)).astype(np.float32)   # deg >= 1 (self loop)

    blk, slot = _balance_nodes(deg, cfg.NBT)

    ecore = blk[cols] // cfg.NBLK
    ebn = blk[cols] % cfg.NBLK
    # per (core, local block) edge counts -> shared chunk schedule
    counts = np.zeros((N_CORES, cfg.NBLK), np.int64)
    for c in range(N_CORES):
        counts[c] = np.bincount(ebn[ecore == c], minlength=cfg.NBLK)
    kj = np.maximum((counts.max(0) + P - 1) // P, 1)
    c_tot = int(kj.sum())
    c_pad = ((c_tot + GRP - 1) // GRP) * GRP
    kj[-1] += c_pad - c_tot
    cbase = np.concatenate([[0], np.cumsum(kj)[:-1]])

    # schedule entries: (block j, start, stop, global chunk, span)
    sched = []
    for j in range(cfg.NBLK):
        t = 0
        while t < kj[j]:
            c0 = cbase[j] + t
            span = 1
            if use_dr and t + 1 < kj[j] and (c0 // GRP) == ((c0 + 1) // GRP):
                span = 2
            sched.append((j, t == 0, t + span == kj[j], int(c0), span))
            t += span

    # per-core streams
    idxs_all, s_all = [], []
    for c in range(N_CORES):
        sel = ecore == c
        r, bn, sl = rows[sel], ebn[sel], slot[cols[sel]]
        order = np.lexsort((r, bn))          # by (block, src)
        r, bn, sl = r[order], bn[order], sl[order]
        starts = np.searchsorted(bn, np.arange(cfg.NBLK), side='left')
        pos = np.arange(len(r)) - starts[bn]
        chunk = cbase[bn] + pos // P
        p128 = pos % P
        srcs = np.zeros((c_pad, P), np.int16)
        srcs[chunk, p128] = r.astype(np.int16)
        smat = np.zeros((c_pad, P, P), np.float32)
        smat[chunk, p128, sl] = 1.0
        idxs_all.append(srcs)
        s_all.append(smat)

    # per-core dinv-by-dest + node lists for Mpool
    dinv_t = np.zeros((N_CORES, P, cfg.NBLK), np.float32)
    nodes = np.full((N_CORES, cfg.NBLK, P), -1, np.int64)
    for nd in range(cfg.N):
        c, j, p = blk[nd] // cfg.NBLK, blk[nd] % cfg.NBLK, slot[nd]
        dinv_t[c, p, j] = dinv[nd]
        nodes[c, j, p] = nd
    return sched, c_pad, idxs_all, s_all, dinv, dinv_t, nodes


def _wrap_idxs(srcs):
    """[C, 128] int16 -> wrapped [128, C*8] (idx q at [q%16 + 16*rep, q//16])."""
    flat = srcs.reshape(-1)
    w = flat.reshape(-1, 16).T
    return np.tile(w, (8, 1)).astype(np.int16)


def _mpool(cfg, batch, nodes_c, dt):
    """[128, NBLK, B] folding 1/cnt at each (slot, block) position."""
    batch = np.asarray(batch).astype(np.int64)
    cnt = np.bincount(batch, minlength=cfg.B).astype(np.float64)
    cinv = (1.0 / np.maximum(cnt, 1.0)).astype(np.float32)
    m = np.zeros((P, cfg.NBLK, cfg.B), np.float32)
    for j in range(cfg.NBLK):
        nd = nodes_c[j]
        ok = nd >= 0
        m[ok, j, batch[nd[ok]]] = cinv[batch[nd[ok]]]
    return m.astype(dt)


def _preprocess(inputs, cfg, gdt, wdt, use_dr):
    import ml_dtypes
    bf16 = ml_dtypes.bfloat16
    meta = {}
    shared = {}

    for br in (1, 2):
        sched, c_pad, idxs, smats, dinv, dinv_t, nodes = _edge_plan(
            cfg, inputs[f'pro{br}_edge_index'], use_dr)
        meta[f'sched{br}'] = sched
        meta[f'cpad{br}'] = c_pad
        x = np.asarray(inputs[f'pro{br}_x'], np.float32)
        shared[f'xs{br}'] = (x * dinv[:, None]).astype(gdt)
        shared[f'_idxs{br}'] = idxs
        shared[f'_smats{br}'] = smats
        shared[f'_dinvt{br}'] = dinv_t
        shared[f'_nodes{br}'] = nodes
        W = np.asarray(inputs[f'W_g{br}'], np.float32) * W_SCALE
        shared[f'Wg{br}'] = np.ascontiguousarray(
            W.reshape(cfg.KC, P, cfg.F)).astype(wdt)
        shared[f'bg{br}'] = np.tile(
            np.asarray(inputs[f'b_g{br}'], np.float32)[None, :], (P, 1))

    def colv(v, n):
        return np.asarray(v, np.float32).reshape(n, 1)

    for br in (1, 2):
        wpf = np.asarray(inputs[f'W_pf{br}'], np.float32)     # [F, 128]
        shared[f'W_pf{br}'] = np.ascontiguousarray(
            wpf.reshape(cfg.KC, P, P)).astype(bf16)
        shared[f'b_pf{br}'] = colv(inputs[f'b_pf{br}'], P)
    shared['W_fc1'] = np.ascontiguousarray(
        np.asarray(inputs['W_fc1'], np.float32).reshape(2, P, 256)).astype(bf16)
    shared['W_fc2'] = np.ascontiguousarray(
        np.asarray(inputs['W_fc2'], np.float32).reshape(2, P, 64)).astype(bf16)
    shared['b_fc1'] = np.asarray(inputs['b_fc1'], np.float32).reshape(2, P, 1)
    shared['b_fc2'] = colv(inputs['b_fc2'], 64)
    wo = np.zeros((2, P, 1), np.float32)
    wo[0, 0:64, 0] = np.asarray(inputs['W_out'], np.float32)[0:64, 0]
    wo[1, :, 0] = np.asarray(inputs['W_out'], np.float32)[64:192, 0]
    shared['W_out'] = wo.astype(bf16)
    shared['b_out'] = colv(inputs['b_out'], 1)
    shared['id128'] = np.eye(P, dtype=bf16)
    shared['id32d'] = np.eye(32, dtype=np.float32)
    for m in (1, 2):
        shared[f'W_m{m}'] = (np.asarray(inputs[f'W_m{m}'], np.float32)
                             / (2.0 * cfg.LW)).reshape(cfg.LB, cfg.WPB, 64)
        shared[f'b_m{m}'] = colv(inputs[f'b_m{m}'], 64)
        for sf, pre in (('s', 'cs'), ('f', 'cf')):
            w = float(np.asarray(inputs[f'{pre}{m}_w'])[0])
            b = float(np.asarray(inputs[f'{pre}{m}_b'])[0])
            shared[f'scale_{sf}{m}'] = np.full((32, 1), w / cfg.C, np.float32)
            shared[f'bias_{sf}{m}'] = np.full((32, 1), b, np.float32)

    in_maps = []
    for core in range(N_CORES):
        m = {k: v for k, v in shared.items() if not k.startswith('_')}
        for br in (1, 2):
            m[f'idx{br}'] = _wrap_idxs(shared[f'_idxs{br}'][core])
            sm = shared[f'_smats{br}'][core]          # [C, 128, 128]
            n_grp = sm.shape[0] // GRP
            m[f'smat{br}'] = np.ascontiguousarray(
                sm.reshape(n_grp, GRP, P, P).transpose(0, 2, 1, 3)).astype(gdt)
            m[f'dinv{br}'] = shared[f'_dinvt{br}'][core] / W_SCALE
            m[f'mp{br}'] = _mpool(cfg, inputs[f'pro{br}_batch'],
                                  shared[f'_nodes{br}'][core], bf16)
        gsel = slice(core * cfg.GPB, (core + 1) * cfg.GPB)
        def maslay(a):
            # [GPB, C, L] -> [LB*GPB, C, LBS]: partition lb*GPB+g = (graph g,
            # l-block lb), matching the on-chip masif tile layout.
            a = np.asarray(a, np.float32)[gsel]
            a = a.reshape(cfg.GPB, cfg.C, cfg.LB, cfg.LBS)
            return np.ascontiguousarray(a.transpose(2, 0, 1, 3).reshape(
                cfg.LB * cfg.GPB, cfg.C, cfg.LBS))
        for mi, names in ((1, ('mas1_straight', 'mas1_flipped')),
                          (2, ('mas2_straight', 'mas2_flipped'))):
            m[f'mas{mi}s'] = maslay(inputs[names[0]])
            m[f'mas{mi}f'] = maslay(inputs[names[1]])
        mk = np.zeros((P, cfg.B), np.float32)
        mk[:, core * cfg.GPB:(core + 1) * cfg.GPB] = 1.0
        m['gmask'] = mk
        in_maps.append(m)
    return meta, in_maps


# ---------------------------------------------------------------- program
def _build(cfg, meta, gdt_np, wdt_np, use_dr, nq):
    import concourse.bass as bass
    import concourse.bacc as bacc
    import concourse.mybir as mybir
    import concourse.tile as tile
    from concourse.masks import make_identity

    dt = mybir.dt
    gdt = dt.from_np(np.dtype(gdt_np))
    wdt = dt.from_np(np.dtype(wdt_np))
    f32 = dt.float32
    bf16 = dt.bfloat16
    AF = mybir.ActivationFunctionType
    OP = mybir.AluOpType
    DR = mybir.MatmulPerfMode.DoubleRow

    nc = bacc.Bacc("TRN2", target_bir_lowering=False, debug=False,
                   enable_asserts=False, num_devices=N_CORES,
                   num_swdge_queues=max(nq, 1))

    def din(name, shape, d):
        return nc.dram_tensor(name, list(shape), d, kind="ExternalInput")

    xs = {br: din(f'xs{br}', (cfg.N, cfg.F), gdt) for br in (1, 2)}
    Wg = {br: din(f'Wg{br}', (cfg.KC, P, cfg.F), wdt) for br in (1, 2)}
    bg = {br: din(f'bg{br}', (P, cfg.F), f32) for br in (1, 2)}
    idx = {br: din(f'idx{br}', (P, meta[f'cpad{br}'] * 8), dt.int16)
           for br in (1, 2)}
    smat = {br: din(f'smat{br}', (meta[f'cpad{br}'] // GRP, P, GRP, P), gdt)
            for br in (1, 2)}
    dinv = {br: din(f'dinv{br}', (P, cfg.NBLK), f32) for br in (1, 2)}
    mp = {br: din(f'mp{br}', (P, cfg.NBLK, cfg.B), bf16) for br in (1, 2)}
    gmask = din('gmask', (P, cfg.B), f32)
    mas = {(mi, sf): din(f'mas{mi}{sf}', (cfg.LB * cfg.GPB, cfg.C, cfg.LBS),
                         f32)
           for mi in (1, 2) for sf in 'sf'}
    w_pf = {br: din(f'W_pf{br}', (cfg.KC, P, P), bf16) for br in (1, 2)}
    b_pf = {br: din(f'b_pf{br}', (P, 1), f32) for br in (1, 2)}
    w_fc1 = din('W_fc1', (2, P, 256), bf16)
    w_fc2 = din('W_fc2', (2, P, 64), bf16)
    b_fc1 = din('b_fc1', (2, P, 1), f32)
    b_fc2 = din('b_fc2', (64, 1), f32)
    w_out = din('W_out', (2, P, 1), bf16)
    b_out = din('b_out', (1, 1), f32)
    w_m = {mi: din(f'W_m{mi}', (cfg.LB, cfg.WPB, 64), f32) for mi in (1, 2)}
    b_m = {mi: din(f'b_m{mi}', (64, 1), f32) for mi in (1, 2)}
    msc = {(mi, sf, kind): din(f'{kind}_{sf}{mi}', (32, 1), f32)
           for mi in (1, 2) for sf in 'sf' for kind in ('scale', 'bias')}

    id128_d = din('id128', (P, P), bf16)
    id32_d = din('id32d', (32, 32), f32)

    out_t = nc.dram_tensor('out', [1, cfg.B], f32, kind="ExternalOutput")

    PF = cfg.KC * cfg.B              # pooledT cols per branch (256)

    with tile.TileContext(nc) as tc:
        with tc.tile_pool(name="const", bufs=1) as cst, \
             tc.tile_pool(name="idxp", bufs=3) as idxp, \
             tc.tile_pool(name="gat", bufs=4) as gatp, \
             tc.tile_pool(name="sld", bufs=3) as sldp, \
             tc.tile_pool(name="sb", bufs=2) as sbp, \
             tc.tile_pool(name="aggps", bufs=2, space="PSUM") as aggps, \
             tc.tile_pool(name="tps", bufs=2, space="PSUM") as tps, \
             tc.tile_pool(name="hps", bufs=1, space="PSUM") as hps, \
             tc.tile_pool(name="small", bufs=2) as smp, \
             tc.tile_pool(name="dram", bufs=1, space="DRAM") as drp:
            smps = tps

            def load(pool, src_ap, shape, d, name):
                # const loads go on the ACT HWDGE ring so the sync ring
                # serves the gather-critical idx/S loads from t=0
                t = pool.tile(list(shape), d, tag=name)
                nc.scalar.dma_start(out=t[:], in_=src_ap)
                return t

            # ---------------- constants
            wg_sb = {br: load(cst, Wg[br].ap().transpose([1, 0, 2]),
                              (P, cfg.KC, cfg.F), wdt, f'wg{br}')
                     for br in (1, 2)}
            bg_sb = {br: load(cst, bg[br][:, :], (P, cfg.F), f32, f'bg{br}')
                     for br in (1, 2)}
            dinv_sb = {br: load(cst, dinv[br][:, :], (P, cfg.NBLK), f32,
                                f'dinv{br}') for br in (1, 2)}
            mp_sb = {br: load(cst, mp[br][:, :, :], (P, cfg.NBLK, cfg.B),
                              bf16, f'mp{br}') for br in (1, 2)}
            gmask_sb = load(cst, gmask[:, :], (P, cfg.B), f32, 'gmask')
            id_bf = load(cst, id128_d[:, :], (P, P), bf16, 'idbf')
            id32 = load(cst, id32_d[:, :], (32, 32), f32, 'id32')

            wpf_sb = {br: load(cst, w_pf[br].ap().transpose([1, 0, 2]),
                               (P, cfg.KC, P), bf16, f'wpf{br}')
                      for br in (1, 2)}
            bpf_sb = {br: load(cst, b_pf[br][:, :], (P, 1), f32, f'bpf{br}')
                      for br in (1, 2)}
            wfc1_sb = load(cst, w_fc1.ap().transpose([1, 0, 2]),
                           (P, 2, 256), bf16, 'wfc1')
            wfc2_sb = load(cst, w_fc2.ap().transpose([1, 0, 2]),
                           (P, 2, 64), bf16, 'wfc2')
            bfc1_sb = load(cst, b_fc1.ap().transpose([1, 0, 2]),
                           (P, 2, 1), f32, 'bfc1')
            bfc2_sb = load(cst, b_fc2[:, :], (64, 1), f32, 'bfc2')
            wout_sb = load(cst, w_out.ap().transpose([1, 0, 2]),
                           (P, 2, 1), bf16, 'wout')
            bout_sb = load(cst, b_out[:, :], (1, 1), f32, 'bout')
            wm_sb = {mi: load(cst, w_m[mi].ap().transpose([1, 0, 2]),
                              (cfg.WPB, cfg.LB, 64), f32, f'wm{mi}')
                     for mi in (1, 2)}
            bm_sb = {mi: load(cst, b_m[mi][:, :], (64, 1), f32, f'bm{mi}')
                     for mi in (1, 2)}
            msc_sb = {k: load(cst, v[:, :], (32, 1), f32, f'msc{k}')
                      for k, v in msc.items()}
            poolacc = {br: cst.tile([P, PF], f32, tag=f'poolacc{br}',
                                    name=f'poolacc{br}')
                       for br in (1, 2)}
            masif_cc = cst.tile([P, cfg.B], f32, tag='masifcc')

            # ---------------- masif (both branches) -> masif_cc [128, B]
            def masif_block(mi):
                frag = None
                for sf in 'sf':
                    src = mas[(mi, sf)]
                    t = smp.tile([32, cfg.C, cfg.LBS], f32, tag='masload')
                    nc.scalar.dma_start(out=t[:], in_=src.ap()[:, :, :])
                    red = smp.tile([32, cfg.LBS], f32, tag='masred')
                    nc.vector.tensor_reduce(
                        out=red[:], in_=t[:].transpose([0, 2, 1]),
                        axis=mybir.AxisListType.X, op=OP.add)
                    act = smp.tile([32, cfg.LBS], f32, tag='masact')
                    nc.scalar.activation(
                        act[:], red[:], AF.Relu,
                        bias=msc_sb[(mi, sf, 'bias')][:, 0:1],
                        scale=msc_sb[(mi, sf, 'scale')][:, 0:1])
                    ws = smp.tile([32, cfg.WPB], f32, tag='masws')
                    nc.vector.tensor_reduce(
                        out=ws[:],
                        in_=act[:].rearrange("p (w l) -> p w l", l=cfg.LW),
                        axis=mybir.AxisListType.X, op=OP.add)
                    if frag is None:
                        frag = ws
                    else:
                        frag2 = smp.tile([32, cfg.WPB], f32, tag='masfrag')
                        nc.vector.tensor_add(out=frag2[:], in0=frag[:],
                                             in1=ws[:])
                        frag = frag2
                ps_t = smps.tile([cfg.WPB, 32], f32, space="PSUM", tag='tp')
                nc.tensor.transpose(out=ps_t[:], in_=frag[:], identity=id32[:])
                fragT = smp.tile([cfg.WPB, 32], f32, tag='masfragT')
                nc.scalar.activation(fragT[:], ps_t[:], AF.Identity)
                fragTc = fragT[:].rearrange("k (lb g) -> k lb g", g=cfg.GPB)
                m_ps = smps.tile([64, cfg.GPB], f32, space="PSUM",
                                 tag='tp')
                for lb in range(cfg.LB):
                    nc.tensor.matmul(
                        m_ps[:], lhsT=wm_sb[mi][:, lb, :], rhs=fragTc[:, lb, :],
                        start=(lb == 0), stop=(lb == cfg.LB - 1))
                m_fm = smp.tile([64, cfg.GPB], f32, tag='masfm')
                nc.scalar.activation(m_fm[:], m_ps[:], AF.Identity,
                                     bias=bm_sb[mi][:, 0:1])
                nc.vector.tensor_tensor(
                    out=masif_cc[(mi - 1) * 64:mi * 64, :].rearrange(
                        "p (s g) -> p s g", g=cfg.GPB),
                    in0=m_fm[:, None, :].to_broadcast(
                        [64, N_CORES, cfg.GPB]),
                    in1=gmask_sb[0:64, :].rearrange(
                        "p (s g) -> p s g", g=cfg.GPB),
                    op=OP.mult)

            for mi in (1, 2):
                masif_block(mi)
            masif_bf = smp.tile([P, cfg.B], bf16, tag='masifbf')
            nc.vector.tensor_copy(out=masif_bf[:], in_=masif_cc[:])

            # ---------------- GCN branches
            CCN = 2 * PF + cfg.B
            bounce_in = drp.tile([P, CCN], bf16, tag='ccin')
            bounce_out = drp.tile([P, CCN], bf16, tag='ccout')

            def block_tail(br, j, agg_ps):
                agg_sb = sbp.tile([P, cfg.F], bf16, tag='aggsb')
                nc.scalar.activation(agg_sb[:], agg_ps[:], AF.Identity,
                                     scale=dinv_sb[br][:, j:j + 1])
                aggT_ps = tps.tile([P, cfg.KC, P], bf16, space="PSUM",
                                   tag='tp')
                for kf in range(cfg.KC):
                    nc.tensor.transpose(
                        out=aggT_ps[:, kf, :],
                        in_=agg_sb[:, kf * P:(kf + 1) * P],
                        identity=id_bf[:])
                aggT_sb = sbp.tile([P, cfg.KC, P], wdt, tag='aggTsb')
                nc.vector.tensor_copy(out=aggT_sb[:], in_=aggT_ps[:])
                h_ps = hps.tile([P, cfg.F], f32, space="PSUM", tag='h')
                for half in range(2):
                    hs = slice(half * 512, (half + 1) * 512)
                    if use_dr:
                        for tpr in range(cfg.KC // 2):
                            nc.tensor.matmul(
                                h_ps[:, hs],
                                lhsT=aggT_sb[:, 2 * tpr:2 * tpr + 2, :],
                                rhs=wg_sb[br][:, 2 * tpr:2 * tpr + 2, hs],
                                perf_mode=DR,
                                start=(tpr == 0), stop=(tpr == cfg.KC // 2 - 1))
                    else:
                        for kf in range(cfg.KC):
                            nc.tensor.matmul(
                                h_ps[:, hs],
                                lhsT=aggT_sb[:, kf, :],
                                rhs=wg_sb[br][:, kf, hs],
                                start=(kf == 0), stop=(kf == cfg.KC - 1))
                h_sb = sbp.tile([P, cfg.F], bf16, tag='hsb')
                nc.vector.tensor_add(out=h_sb[:], in0=h_ps[:],
                                     in1=bg_sb[br][:, :])
                nc.scalar.activation(h_sb[:], h_sb[:], AF.Lrelu, alpha=0.01)
                pool_ps = tps.tile([P, cfg.KC, cfg.B], f32, space="PSUM",
                                   tag='tp')
                for kf in range(cfg.KC):
                    nc.tensor.matmul(
                        pool_ps[:, kf, :],
                        lhsT=h_sb[:, kf * P:(kf + 1) * P],
                        rhs=mp_sb[br][:, j, :], start=True, stop=True)
                if j == 0:
                    nc.vector.tensor_copy(
                        out=poolacc[br][:],
                        in_=pool_ps[:].rearrange("p a b -> p (a b)"))
                else:
                    nc.vector.tensor_add(
                        out=poolacc[br][:], in0=poolacc[br][:],
                        in1=pool_ps[:].rearrange("p a b -> p (a b)"))

            for br in (1, 2):
                cpad = meta[f'cpad{br}']
                sched = meta[f'sched{br}']
                n_grp = cpad // GRP
                agg_ps = None
                si = 0
                for g in range(n_grp):
                    idx_t = idxp.tile([P, GRP * 8], dt.int16, tag='idx')
                    nc.sync.dma_start(
                        out=idx_t[:],
                        in_=idx[br][:, g * GRP * 8:(g + 1) * GRP * 8])
                    gat_t = gatp.tile([P, GRP, cfg.F], gdt, tag='gat')
                    nc.gpsimd.dma_gather(
                        out_ap=gat_t[:], in_ap=xs[br][:, :], idxs_ap=idx_t[:],
                        num_idxs=GRP * P, num_idxs_reg=GRP * P,
                        elem_size=cfg.F, queue_num=(g % nq))
                    s_t = sldp.tile([P, GRP, P], gdt, tag='sld')
                    nc.sync.dma_start(out=s_t[:], in_=smat[br][g, :, :, :])
                    glo, ghi = g * GRP, (g + 1) * GRP
                    while si < len(sched) and sched[si][3] < ghi:
                        j, st, sp, c0, span = sched[si]
                        assert c0 >= glo and c0 + span <= ghi
                        i = c0 - glo
                        if st:
                            agg_ps = aggps.tile([P, cfg.F], f32, space="PSUM",
                                                tag='agg')
                        for half in range(2):
                            hs = slice(half * 512, (half + 1) * 512)
                            if span == 2:
                                nc.tensor.matmul(
                                    agg_ps[:, hs],
                                    lhsT=s_t[:, i:i + 2, :],
                                    rhs=gat_t[:, i:i + 2, hs],
                                    perf_mode=DR, start=st, stop=sp)
                            else:
                                nc.tensor.matmul(
                                    agg_ps[:, hs],
                                    lhsT=s_t[:, i, :],
                                    rhs=gat_t[:, i, hs],
                                    start=st, stop=sp)
                        if sp:
                            block_tail(br, j, agg_ps)
                        si += 1
                assert si == len(sched)
                pbf = smp.tile([P, PF], bf16, tag=f'poolb{br}',
                               name=f'poolb{br}')
                nc.vector.tensor_copy(out=pbf[:], in_=poolacc[br][:])
                nc.sync.dma_start(
                    out=bounce_in[:, (br - 1) * PF:br * PF], in_=pbf[:])

            # ---------------- single collective after all compute
            nc.sync.dma_start(
                out=bounce_in[:, 2 * PF:], in_=masif_bf[:])
            nc.gpsimd.collective_compute(
                "AllReduce", OP.add,
                replica_groups=[list(range(N_CORES))],
                ins=[bounce_in[:].opt()], outs=[bounce_out[:].opt()])

            # ---------------- head (replicated on all cores)
            gm_bf = smp.tile([P, CCN], bf16, tag='poolgm')
            nc.sync.dma_start(out=gm_bf[:], in_=bounce_out[:, :])
            pooled_bf = {br: gm_bf[:, (br - 1) * PF:br * PF]
                         for br in (1, 2)}
            masif_hd = gm_bf[:, 2 * PF:]

            pbv = {br: pooled_bf[br].rearrange(
                "p (a b) -> p a b", b=cfg.B) for br in (1, 2)}
            x12 = {}
            for br in (1, 2):
                xps = smps.tile([P, cfg.B], f32, space="PSUM", tag='tp')
                for kf in range(cfg.KC):
                    nc.tensor.matmul(
                        xps[:], lhsT=wpf_sb[br][:, kf, :],
                        rhs=pbv[br][:, kf, :],
                        start=(kf == 0), stop=(kf == cfg.KC - 1))
                xsb = smp.tile([P, cfg.B], bf16, tag=f'x{br}')
                nc.scalar.activation(xsb[:], xps[:], AF.Lrelu,
                                     bias=bpf_sb[br][:, 0:1], alpha=0.01)
                x12[br] = xsb
            xc1 = {}
            for mh in range(2):
                cps = smps.tile([P, cfg.B], f32, space="PSUM", tag='tp')
                for k2 in range(2):
                    nc.tensor.matmul(
                        cps[:], lhsT=wfc1_sb[:, k2, mh * P:(mh + 1) * P],
                        rhs=x12[k2 + 1][:], start=(k2 == 0), stop=(k2 == 1))
                xcs = smp.tile([P, cfg.B], bf16, tag=f'xc{mh}')
                nc.scalar.activation(xcs[:], cps[:], AF.Lrelu,
                                     bias=bfc1_sb[:, mh, 0:1], alpha=0.01)
                xc1[mh] = xcs
            c2ps = smps.tile([64, cfg.B], f32, space="PSUM", tag='tp')
            for k2 in range(2):
                nc.tensor.matmul(c2ps[:], lhsT=wfc2_sb[:, k2, :],
                                 rhs=xc1[k2][:], start=(k2 == 0),
                                 stop=(k2 == 1))
            xc = smp.tile([64, cfg.B], bf16, tag='xcf')
            nc.scalar.activation(xc[:], c2ps[:], AF.Lrelu,
                                 bias=bfc2_sb[:, 0:1], alpha=0.01)

            ops = smps.tile([1, cfg.B], f32, space="PSUM", tag='tp')
            nc.tensor.matmul(ops[:], lhsT=wout_sb[0:64, 0, :], rhs=xc[:],
                             start=True, stop=False)
            nc.tensor.matmul(ops[:], lhsT=wout_sb[:, 1, :], rhs=masif_hd,
                             start=False, stop=True)
            res = smp.tile([1, cfg.B], f32, tag='res')
            nc.scalar.activation(res[:], ops[:], AF.Sigmoid,
                                 bias=bout_sb[:, 0:1])
            nc.sync.dma_start(out=out_t[:, :], in_=res[:])

    nc.compile()
    return nc


# ---------------------------------------------------------------- entry
_CACHE = {}


def _run(inputs, cfg, gdt=None, wdt=None, use_dr=None, nq=None,
         trace=False, tmpdir=None):
    import ml_dtypes
    from concourse import bass_utils
    gdt = np.dtype(getattr(ml_dtypes, CFG_GDT if gdt is None else gdt))
    wdt = np.dtype(getattr(ml_dtypes, CFG_WDT if wdt is None else wdt))
    use_dr = CFG_DR if use_dr is None else use_dr
    nq = CFG_NQ if nq is None else nq
    meta, in_maps = _preprocess(inputs, cfg, gdt, wdt, use_dr)
    key = (cfg.N, cfg.F, meta['cpad1'], meta['cpad2'],
           tuple(meta['sched1']), tuple(meta['sched2']),
           gdt.name, wdt.name, use_dr, nq)
    if key not in _CACHE:
        _CACHE.clear()
        _CACHE[key] = _build(cfg, meta, gdt, wdt, use_dr, nq)
    nc = _CACHE[key]
    res = bass_utils.run_bass_kernel_spmd(
        nc, in_maps, core_ids=list(range(N_CORES)), trace=trace,
        tmpdir=tmpdir)
    out = np.asarray(res.results[0]['out'], np.float32).reshape(cfg.B, 1)
    return out, res


def kernel(**inputs) -> np.ndarray:
    cfg = _Cfg()
    out, _ = _run(inputs, cfg)
    return out
